# revision 2
# baseline (speedup 1.0000x reference)
"""Trainium2 Bass kernel for AngularTerms: out[p, a*8+s] = 2*f1[p,s]*f2[p,a]*fcj[p].

Self-contained: hardcodes shapes for vectors12 (2, 2000000, 3) f32 -> (2000000, 64) f32.
Data-parallel over the pair axis P across 8 NeuronCores; no collectives.

Math (per pair p, with v0, v1 the two displacement vectors):
  d_i   = |v_i|
  c     = dot(v0,v1) / (d0*d1)                (clamp is a no-op for this data)
  x     = 0.95*c = cos(theta);  y = sqrt(1 - x^2) = sin(theta)
  f1[s] = ((1 + x*cos(ShfZ_s) + y*sin(ShfZ_s)) / 2) ** 32     (angle-addition; no arccos)
  f2[a] = exp(-8*(h - ShfA_a)^2),  h = (d0+d1)/2
  fcj   = prod_i (0.5*cos(pi*d_i/3.5)+0.5) = (sin(pi/2 - pi*d0/7) * sin(pi/2 - pi*d1/7))^2
  out[p, a*8+s] = 2 * f1[s] * f2[a] * fcj

Engine allocation (v2):
  - ACT (scalar): Square/Sqrt/Sin/Ln/Exp.  The f2 broadcast-expansion is only
    pairwise-duplicated (F2d[n,a,j], j in {0,1}) instead of 8-wide: the DVE 2x
    packed mode only needs innermost runs of 2 identical fp16 values, so the
    final outer-product multiply runs in 4 passes over s-pairs, each with the
    dense F2d as one operand. This cuts ACT's expansion cost 4x vs expanding
    to the full [n,a,s] tensor.
  - U^2 runs in phase A (Square binds to the sqrt table set), carried as fp16,
    so phase C issues no Square -> no ACT table-set thrash (3 loads/group).
  - GPSIMD (idle otherwise) computes the three broadcast tensor_tensor ops:
    A8 = CA*c, B8 = SA*y, U = s01 - 2*ShfA.
  - DVE: reductions, small per-pair chain, T8=A8+B8, fcj fold (2x via
    pair-duplicated qq2), and the 4 final outer-product multiplies (2x fp16).
"""
import sys

sys.path.insert(0, "/opt/trn_rl_repo")

import numpy as np
from contextlib import ExitStack

import concourse.bass as bass
import concourse.tile as tile
from concourse import bacc, mybir
from concourse.bass_utils import run_bass_kernel_spmd

F32 = mybir.dt.float32
F16 = mybir.dt.float16
AL = mybir.AluOpType
AF = mybir.ActivationFunctionType

P_TOTAL = 2_000_000
NCORES = 8
P_CORE = P_TOTAL // NCORES      # 250,000
N = 196                          # pairs per partition per tile
T = 10                           # tiles per core
P_PAD = 128 * N * T              # 250,880
CUTOFF = 3.5
NGROUPS = 2                      # table-set rounds; halves the carried-state SBUF

SHFA = np.array([0.9, 1.225, 1.55, 1.875, 2.2, 2.525, 2.85, 3.175], np.float32)
SHFZ = np.array([0.19634954, 0.58904862, 0.9817477, 1.37444679,
                 1.76714587, 2.15984495, 2.55254403, 2.94524311], np.float32)

_CACHE: dict = {}


def _build_nc(N=N, T=T, ngroups=NGROUPS, out_bufs=2,
              gps_a8=True, gps_b8=True, gps_u=True):
    P_PAD = 128 * N * T
    TILE_PAIRS = 128 * N
    assert T % ngroups == 0
    TG = T // ngroups
    nc = bacc.Bacc()
    vec = nc.declare_dram_parameter("vectors12", [2, P_PAD, 3], F32, isOutput=False)
    cst = nc.declare_dram_parameter("cst", [128, 24], F32, isOutput=False)
    out = nc.declare_dram_parameter("out", [P_PAD, 64], F16, isOutput=True)

    act = nc.scalar.activation

    with tile.TileContext(nc) as tc, ExitStack() as ctx:
        const = ctx.enter_context(tc.tile_pool(name="const", bufs=1))
        carp = ctx.enter_context(tc.tile_pool(name="car", bufs=1))
        pA = ctx.enter_context(tc.tile_pool(name="pA", bufs=2))
        tmpA = ctx.enter_context(tc.tile_pool(name="tmpA", bufs=3))
        pU = ctx.enter_context(tc.tile_pool(name="pU", bufs=2))
        pB = ctx.enter_context(tc.tile_pool(name="pB", bufs=2))
        pC = ctx.enter_context(tc.tile_pool(name="pC", bufs=2))
        pF2 = ctx.enter_context(tc.tile_pool(name="pF2", bufs=2))
        outp = ctx.enter_context(tc.tile_pool(name="outp", bufs=out_bufs))

        cstT = const.tile([128, 24], F32)
        nc.sync.dma_start(cstT[:], cst[:])
        CA = cstT[:, 0:8]     # 0.475*cos(ShfZ)
        SA = cstT[:, 8:16]    # 0.5*sin(ShfZ)
        A2 = cstT[:, 16:24]   # 2*ShfA

        def const_scalar(val, name):
            t = const.tile([128, 1], F32, tag=name)
            nc.vector.memset(t[:], float(val))
            return t[:]

        b_pi2 = const_scalar(np.pi / 2, "pi2")
        b_half = const_scalar(0.5, "half")
        b_ln2 = const_scalar(float(np.log(2.0)), "ln2")
        b_one = const_scalar(1.0, "one")

        # carried per-tile state across phases (within one group):
        #   f32: d0,d1 (2N) | c (N) | y (N)      fp16: G=(s01-2A)^2 (8N) | qq2 (2N)
        CAR_F32 = 4 * N
        CAR_F16 = 10 * N

        for g in range(ngroups):
            carf = carp.tile([128, CAR_F32 * TG], F32, tag="carf")
            carh = carp.tile([128, CAR_F16 * TG], F16, tag="carh")

            def slots(tl):
                b32 = tl * CAR_F32
                b16 = tl * CAR_F16
                d_sl = carf[:, b32: b32 + 2 * N]
                c_sl = carf[:, b32 + 2 * N: b32 + 3 * N]
                y_sl = carf[:, b32 + 3 * N: b32 + 4 * N]
                g_sl = carh[:, b16: b16 + 8 * N]
                qq_sl = carh[:, b16 + 8 * N: b16 + 10 * N]
                return d_sl, c_sl, y_sl, g_sl, qq_sl

            # ---------- Phase A: squares, norms, c, y, G (sqrt table set) ----
            for tl in range(TG):
                base = (g * TG + tl) * TILE_PAIRS
                d_sl, c_sl, y_sl, g_sl, _ = slots(tl)

                VV = pA.tile([128, 6 * N], F32, tag="VV")
                nc.sync.dma_start(
                    VV[:, : 3 * N],
                    vec[0, base: base + TILE_PAIRS, :].rearrange("(p n) c -> p (n c)", p=128),
                )
                nc.sync.dma_start(
                    VV[:, 3 * N:],
                    vec[1, base: base + TILE_PAIRS, :].rearrange("(p n) c -> p (n c)", p=128),
                )
                SQ = pA.tile([128, 6 * N], F32, tag="SQ")
                act(SQ[:], VV[:], AF.Square)

                PR = pA.tile([128, 3 * N], F32, tag="PR")
                nc.vector.tensor_tensor(PR[:], VV[:, : 3 * N], VV[:, 3 * N:], AL.mult)

                PR3 = PR[:].rearrange("p (n c) -> p n c", c=3)
                dotv = tmpA.tile([128, N], F32, tag="dotv")
                nc.vector.tensor_tensor(dotv[:], PR3[:, :, 0], PR3[:, :, 1], AL.add)
                nc.vector.tensor_tensor(dotv[:], dotv[:], PR3[:, :, 2], AL.add)

                SQ4 = SQ[:].rearrange("p (i n c) -> p i n c", i=2, c=3)
                D2 = pA.tile([128, 2 * N], F32, tag="D2")
                D2v = D2[:].rearrange("p (i n) -> p i n", i=2)
                nc.vector.tensor_tensor(D2v, SQ4[:, :, :, 0], SQ4[:, :, :, 1], AL.add)
                nc.vector.tensor_tensor(D2v, D2v, SQ4[:, :, :, 2], AL.add)

                act(d_sl, D2[:], AF.Sqrt)

                s01 = tmpA.tile([128, N], F32, tag="s01")
                nc.vector.tensor_tensor(s01[:], d_sl[:, :N], d_sl[:, N:], AL.add)

                m = tmpA.tile([128, N], F32, tag="m")
                nc.vector.tensor_tensor(m[:], d_sl[:, :N], d_sl[:, N:], AL.mult)
                rm = tmpA.tile([128, N], F32, tag="rm")
                nc.vector.reciprocal_approx_fast(rm[:], m[:])
                nc.vector.tensor_tensor(c_sl, dotv[:], rm[:], AL.mult)

                # cc = -0.9025 c^2 (scale folded);  y = sqrt(cc + 1) = sin(theta)
                cc = tmpA.tile([128, N], F32, tag="cc")
                nc.vector.scalar_tensor_tensor(
                    cc[:], c_sl, -0.9025, c_sl, AL.mult, AL.mult)
                act(y_sl, cc[:], AF.Sqrt, bias=b_one)

                # U = s01 - 2*ShfA_a; G = U^2 (Square binds to the sqrt set,
                # so it must run here, not in phase C)
                U = pU.tile([128, 8 * N], F32, tag="U")
                Uv = U[:].rearrange("p (n a) -> p n a", a=8)
                s01b = s01[:, :, None].to_broadcast([128, N, 8])
                A2b = A2[:, None, :].to_broadcast([128, N, 8])
                if gps_u:
                    nc.gpsimd.tensor_tensor(Uv, s01b, A2b, AL.subtract)
                else:
                    nc.vector.tensor_tensor(Uv, s01b, A2b, AL.subtract)
                act(g_sl, U[:], AF.Square)

            # ---------- Phase B: fcj via sin (trig table set) ---------------
            for tl in range(TG):
                d_sl, _, _, _, qq_sl = slots(tl)
                S12 = pB.tile([128, 2 * N], F32, tag="S12")
                # sin(pi/2 - (pi/7) d) = cos(pi d / 7);  fcj_i = cos^2(pi d_i/7)
                act(S12[:], d_sl, AF.Sin, bias=b_pi2, scale=float(-np.pi / 7))
                q = pB.tile([128, N], F32, tag="q")
                nc.vector.tensor_tensor(q[:], S12[:, :N], S12[:, N:], AL.mult)
                # qq2[n, j] = fcj (duplicated pair) so the F1 fold runs at 2x
                qqv = qq_sl.rearrange("p (n j) -> p n j", j=2)
                qb = q[:, :, None].to_broadcast([128, N, 2])
                nc.vector.tensor_tensor(qqv, qb, qb, AL.mult)

            # ---------- Phase C: f1, f2, outer product (ln+exp set) ---------
            for tl in range(TG):
                base = (g * TG + tl) * TILE_PAIRS
                _, c_sl, y_sl, g_sl, qq_sl = slots(tl)

                A8 = pC.tile([128, 8 * N], F32, tag="A8")
                B8 = pC.tile([128, 8 * N], F32, tag="B8")
                A8v = A8[:].rearrange("p (n s) -> p n s", s=8)
                B8v = B8[:].rearrange("p (n s) -> p n s", s=8)
                cb = c_sl[:, :, None].to_broadcast([128, N, 8])
                yb = y_sl[:, :, None].to_broadcast([128, N, 8])
                CAb = CA[:, None, :].to_broadcast([128, N, 8])
                SAb = SA[:, None, :].to_broadcast([128, N, 8])
                if gps_a8:
                    nc.gpsimd.tensor_tensor(A8v, CAb, cb, AL.mult)
                else:
                    nc.vector.tensor_tensor(A8v, CAb, cb, AL.mult)
                if gps_b8:
                    nc.gpsimd.tensor_tensor(B8v, SAb, yb, AL.mult)
                else:
                    nc.vector.tensor_tensor(B8v, SAb, yb, AL.mult)
                nc.vector.tensor_tensor(A8[:], A8[:], B8[:], AL.add)
                # lt = ln(x*ca + y*sa + 0.5); f1 = exp(32*lt) = t^32
                act(A8[:], A8[:], AF.Ln, bias=b_half)
                F1 = pC.tile([128, 8 * N], F16, tag="F1")
                act(F1[:], A8[:], AF.Exp, scale=32.0)
                # fold fcj: F1 *= qq2 (2x: qq2 read as packed pairs)
                F1v = F1[:].rearrange("p (n h j) -> p n h j", h=4, j=2)
                qqv = qq_sl.rearrange("p (n j) -> p n j", j=2)
                qqb = qqv[:, :, None, :].to_broadcast([128, N, 4, 2])
                nc.vector.tensor_tensor(F1v, F1v, qqb, AL.mult)

                # F2d[n, a, j] = 2*f2[a] duplicated pairwise (j in {0,1})
                F2d = pF2.tile([128, 16 * N], F16, tag="F2d")
                F2dv = F2d[:].rearrange("p (n a j) -> p n a j", a=8, j=2)
                gv = g_sl.rearrange("p (n a) -> p n a", a=8)
                gb = gv[:, :, :, None].to_broadcast([128, N, 8, 2])
                act(F2dv, gb, AF.Exp, bias=b_ln2, scale=-2.0)

                OUT = outp.tile([128, 64 * N], F16, tag="OUT")
                OUTv = OUT[:].rearrange("p (n a s4 j) -> p n a s4 j", a=8, s4=4, j=2)
                F1j = F1[:].rearrange("p (n s4 j) -> p n s4 j", s4=4, j=2)
                for s4 in range(4):
                    F1b = F1j[:, :, None, s4, :].to_broadcast([128, N, 8, 2])
                    nc.vector.tensor_tensor(OUTv[:, :, :, s4, :], F1b, F2dv,
                                            AL.mult)

                nc.sync.dma_start(
                    out[base: base + TILE_PAIRS, :].rearrange("(p n) f -> p (n f)", p=128),
                    OUT[:],
                )

    # The table-load pass greedily binds each activation fn to the FIRST set
    # containing it. Restrict membership so each phase's functions resolve to
    # one set: Square/Sqrt -> sqrt set (phase A), Sin -> trig (phase B),
    # Ln/Exp -> natural_log_exp (phase C). 3 loads per group, no thrash.
    import concourse.bacc as bacc_mod
    from concourse.hw_specs import get_activation_tables as _real_gat
    keep = {"sqrt_and_others", "trig_and_small", "natural_log_exp_and_others"}

    def _gat(arch):
        return {k: (v if k in keep else set()) for k, v in _real_gat(arch).items()}

    bacc_mod.get_activation_tables = _gat
    try:
        nc.compile()
    finally:
        bacc_mod.get_activation_tables = _real_gat
    return nc


def _cst_array() -> np.ndarray:
    row = np.concatenate([
        (0.475 * np.cos(SHFZ)).astype(np.float32),
        (0.5 * np.sin(SHFZ)).astype(np.float32),
        (2.0 * SHFA).astype(np.float32),
    ])
    return np.broadcast_to(row, (128, 24)).copy()


def _run(vectors12: np.ndarray, trace: bool = False):
    if "nc" not in _CACHE:
        _CACHE["nc"] = _build_nc()
    nc = _CACHE["nc"]

    v = np.ascontiguousarray(np.asarray(vectors12, dtype=np.float32))
    pad = np.zeros((2, P_PAD - P_CORE, 3), np.float32)
    pad[:, :, 0] = 1.0  # unit vectors: all downstream math well-defined
    cst = _cst_array()

    in_maps = []
    for i in range(NCORES):
        shard = v[:, i * P_CORE: (i + 1) * P_CORE, :]
        shard = np.concatenate([shard, pad], axis=1)
        in_maps.append({"vectors12": np.ascontiguousarray(shard), "cst": cst})

    res = run_bass_kernel_spmd(nc, in_maps, core_ids=list(range(NCORES)),
                               trace=trace)
    out = np.empty((P_TOTAL, 64), np.float32)
    for i in range(NCORES):
        shard_out = np.asarray(res.results[i]["out"])[:P_CORE]
        out[i * P_CORE: (i + 1) * P_CORE] = shard_out.astype(np.float32)
    return out, res


def kernel(vectors12, EtaA=None, Zeta=None, ShfA=None, ShfZ=None):
    out, _ = _run(vectors12, trace=False)
    return out


# revision 5
# speedup vs baseline: 1.0146x; 1.0146x over previous
"""Trainium2 Bass kernel for AngularTerms: out[p, a*8+s] = 2*f1[p,s]*f2[p,a]*fcj[p].

Self-contained: hardcodes shapes for vectors12 (2, 2000000, 3) f32 -> (2000000, 64) f32.
Data-parallel over the pair axis P across 8 NeuronCores; no collectives.

Math (per pair p, with v0, v1 the two displacement vectors):
  d_i   = |v_i|
  c     = dot(v0,v1) / (d0*d1)                (clamp is a no-op for this data)
  x     = 0.95*c = cos(theta);  y = sqrt(1 - x^2) = sin(theta)
  f1[s] = ((1 + x*cos(ShfZ_s) + y*sin(ShfZ_s)) / 2) ** 32     (angle-addition; no arccos)
  f2[a] = exp(-8*(h - ShfA_a)^2),  h = (d0+d1)/2
  fcj   = prod_i (0.5*cos(pi*d_i/3.5)+0.5) = (sin(pi/2 - pi*d0/7) * sin(pi/2 - pi*d1/7))^2
  out[p, a*8+s] = 2 * f1[s] * f2[a] * fcj

v3 engine allocation (all fp16 on the wide path; DVE runs 2x packed mode):
  - The f2 expansion is pairwise-duplicated (F2d[n,a,j], j in {0,1}): DVE's 2x
    mode only needs innermost runs of 2 identical fp16 values, so the final
    outer product runs as 4 multiplies over s-pairs at full 2x.  This is 4x
    less ACT expansion work than expanding f2 to the full [n,a,s] tensor.
  - G = (s01 - 2*ShfA_a)^2 is computed by 8 narrow ACT Squares (bias = the
    per-a constant, free on ACT), eliminating the DVE broadcast-subtract.
    G lives in phase A so Square stays bound to the sqrt table set.
  - ACT instructions carry a linear dep chain so the scheduler cannot
    interleave phases (table sets never thrash: 3 loads per group).
  - No gpsimd: it shares its SBUF port with the DVE and holds it ~2.5x longer
    per element, slowing concurrent DVE work more than the offload saves.
"""
import sys

sys.path.insert(0, "/opt/trn_rl_repo")

import numpy as np
from contextlib import ExitStack

import concourse.bass as bass
import concourse.tile as tile
from concourse import bacc, mybir
from concourse.bass_utils import run_bass_kernel_spmd

F32 = mybir.dt.float32
F16 = mybir.dt.float16
AL = mybir.AluOpType
AF = mybir.ActivationFunctionType

P_TOTAL = 2_000_000
NCORES = 8
P_CORE = P_TOTAL // NCORES      # 250,000
N = 196                          # pairs per partition per tile
T = 10                           # tiles per core
P_PAD = 128 * N * T              # 250,880
CUTOFF = 3.5
NGROUPS = 2                      # table-set rounds; halves the carried-state SBUF

SHFA = np.array([0.9, 1.225, 1.55, 1.875, 2.2, 2.525, 2.85, 3.175], np.float32)
SHFZ = np.array([0.19634954, 0.58904862, 0.9817477, 1.37444679,
                 1.76714587, 2.15984495, 2.55254403, 2.94524311], np.float32)

_CACHE: dict = {}


def _build_nc(N=N, T=T, ngroups=NGROUPS, out_bufs=2, act_chain=True):
    P_PAD = 128 * N * T
    TILE_PAIRS = 128 * N
    assert T % ngroups == 0
    TG = T // ngroups
    nc = bacc.Bacc()
    vec = nc.declare_dram_parameter("vectors12", [2, P_PAD, 3], F32, isOutput=False)
    cst = nc.declare_dram_parameter("cst", [128, 24], F32, isOutput=False)
    out = nc.declare_dram_parameter("out", [P_PAD, 64], F16, isOutput=True)

    from concourse.bass import _add_dep_helper
    prev_act = [None]

    def act(*args, **kw):
        ins = nc.scalar.activation(*args, **kw)
        if act_chain and prev_act[0] is not None:
            _add_dep_helper(ins.ins, prev_act[0].ins, sync=False,
                            reason="act stream order")
        prev_act[0] = ins
        return ins

    with tile.TileContext(nc) as tc, ExitStack() as ctx:
        const = ctx.enter_context(tc.tile_pool(name="const", bufs=1))
        carp = ctx.enter_context(tc.tile_pool(name="car", bufs=1))
        pA = ctx.enter_context(tc.tile_pool(name="pA", bufs=2))
        tmpA = ctx.enter_context(tc.tile_pool(name="tmpA", bufs=3))
        pB = ctx.enter_context(tc.tile_pool(name="pB", bufs=2))
        pC = ctx.enter_context(tc.tile_pool(name="pC", bufs=2))
        pF2 = ctx.enter_context(tc.tile_pool(name="pF2", bufs=2))
        outp = ctx.enter_context(tc.tile_pool(name="outp", bufs=out_bufs))

        cstT = const.tile([128, 24], F32)
        nc.sync.dma_start(cstT[:], cst[:])
        CA = cstT[:, 0:8]     # 0.475*cos(ShfZ)
        SA = cstT[:, 8:16]    # 0.5*sin(ShfZ)

        def const_scalar(val, name):
            t = const.tile([128, 1], F32, tag=name)
            nc.vector.memset(t[:], float(val))
            return t[:]

        b_pi2 = const_scalar(np.pi / 2, "pi2")
        b_half = const_scalar(0.5, "half")
        b_ln2 = const_scalar(float(np.log(2.0)), "ln2")
        b_one = const_scalar(1.0, "one")
        b_shfa = [const_scalar(-2.0 * float(SHFA[a]), f"shfa{a}") for a in range(8)]

        # carried per-tile state across phases (within one group):
        #   f32: d0,d1 (2N) | c (N) | y (N)
        #   fp16: G (8N, a-major: [a, n]) | qq2 (2N, pair-duplicated fcj)
        CAR_F32 = 4 * N
        CAR_F16 = 10 * N

        for g in range(ngroups):
            carf = carp.tile([128, CAR_F32 * TG], F32, tag="carf")
            carh = carp.tile([128, CAR_F16 * TG], F16, tag="carh")

            def slots(tl):
                b32 = tl * CAR_F32
                b16 = tl * CAR_F16
                d_sl = carf[:, b32: b32 + 2 * N]
                c_sl = carf[:, b32 + 2 * N: b32 + 3 * N]
                y_sl = carf[:, b32 + 3 * N: b32 + 4 * N]
                g_sl = carh[:, b16: b16 + 8 * N]
                qq_sl = carh[:, b16 + 8 * N: b16 + 10 * N]
                return d_sl, c_sl, y_sl, g_sl, qq_sl

            # ---------- Phase A: squares, norms, c, y, G (sqrt table set) ----
            for tl in range(TG):
                base = (g * TG + tl) * TILE_PAIRS
                d_sl, c_sl, y_sl, g_sl, _ = slots(tl)

                VV = pA.tile([128, 6 * N], F32, tag="VV")
                nc.sync.dma_start(
                    VV[:, : 3 * N],
                    vec[0, base: base + TILE_PAIRS, :].rearrange("(p n) c -> p (n c)", p=128),
                )
                nc.sync.dma_start(
                    VV[:, 3 * N:],
                    vec[1, base: base + TILE_PAIRS, :].rearrange("(p n) c -> p (n c)", p=128),
                )
                # W = [v0^2 (3N) | v1^2 (3N) | v0*v1 (3N)], then one tree
                # reduction over the component axis for all three at once
                W = pA.tile([128, 9 * N], F32, tag="W")
                act(W[:, : 6 * N], VV[:], AF.Square)
                nc.vector.tensor_tensor(W[:, 6 * N:], VV[:, : 3 * N], VV[:, 3 * N:],
                                        AL.mult)
                W3 = W[:].rearrange("p (k n c) -> p k n c", k=3, c=3)
                D3 = pA.tile([128, 3 * N], F32, tag="D3")
                D3v = D3[:].rearrange("p (k n) -> p k n", k=3)
                nc.vector.tensor_tensor(D3v, W3[:, :, :, 0], W3[:, :, :, 1], AL.add)
                nc.vector.tensor_tensor(D3v, D3v, W3[:, :, :, 2], AL.add)
                # D3 = [d0^2 | d1^2 | dot]
                act(d_sl, D3[:, : 2 * N], AF.Sqrt)

                s01 = tmpA.tile([128, N], F32, tag="s01")
                nc.vector.tensor_tensor(s01[:], d_sl[:, :N], d_sl[:, N:], AL.add)

                m = tmpA.tile([128, N], F32, tag="m")
                nc.vector.tensor_tensor(m[:], d_sl[:, :N], d_sl[:, N:], AL.mult)
                rm = tmpA.tile([128, N], F32, tag="rm")
                nc.vector.reciprocal_approx_fast(rm[:], m[:])
                nc.vector.tensor_tensor(c_sl, D3[:, 2 * N:], rm[:], AL.mult)

                # cc = -0.9025 c^2 (scale folded);  y = sqrt(cc + 1) = sin(theta)
                cc = tmpA.tile([128, N], F32, tag="cc")
                nc.vector.scalar_tensor_tensor(
                    cc[:], c_sl, -0.9025, c_sl, AL.mult, AL.mult)
                act(y_sl, cc[:], AF.Sqrt, bias=b_one)

                # G[a, n] = (s01 - 2*ShfA_a)^2 via 8 narrow Squares with the
                # per-a constant folded into the free ACT bias
                for a in range(8):
                    act(g_sl[:, a * N: (a + 1) * N], s01[:], AF.Square,
                        bias=b_shfa[a])

            # ---------- Phase B: fcj via sin (trig table set) ---------------
            for tl in range(TG):
                d_sl, _, _, _, qq_sl = slots(tl)
                S12 = pB.tile([128, 2 * N], F32, tag="S12")
                # sin(pi/2 - (pi/7) d) = cos(pi d / 7);  fcj_i = cos^2(pi d_i/7)
                act(S12[:], d_sl, AF.Sin, bias=b_pi2, scale=float(-np.pi / 7))
                q = pB.tile([128, N], F32, tag="q")
                nc.vector.tensor_tensor(q[:], S12[:, :N], S12[:, N:], AL.mult)
                # qq2[n, j] = fcj (duplicated pair) so the F1 fold runs at 2x
                qqv = qq_sl.rearrange("p (n j) -> p n j", j=2)
                qb = q[:, :, None].to_broadcast([128, N, 2])
                nc.vector.tensor_tensor(qqv, qb, qb, AL.mult)

            # ---------- Phase C: f1, f2, outer product (ln+exp set) ---------
            for tl in range(TG):
                base = (g * TG + tl) * TILE_PAIRS
                _, c_sl, y_sl, g_sl, qq_sl = slots(tl)

                A8 = pC.tile([128, 8 * N], F32, tag="A8")
                B8 = pC.tile([128, 8 * N], F32, tag="B8")
                A8v = A8[:].rearrange("p (n s) -> p n s", s=8)
                B8v = B8[:].rearrange("p (n s) -> p n s", s=8)
                cb = c_sl[:, :, None].to_broadcast([128, N, 8])
                yb = y_sl[:, :, None].to_broadcast([128, N, 8])
                CAb = CA[:, None, :].to_broadcast([128, N, 8])
                SAb = SA[:, None, :].to_broadcast([128, N, 8])
                nc.vector.tensor_tensor(A8v, CAb, cb, AL.mult)
                nc.vector.tensor_tensor(B8v, SAb, yb, AL.mult)
                nc.vector.tensor_tensor(A8[:], A8[:], B8[:], AL.add)
                # lt = ln(x*ca + y*sa + 0.5); f1 = exp(32*lt) = t^32
                act(A8[:], A8[:], AF.Ln, bias=b_half)
                F1 = pC.tile([128, 8 * N], F16, tag="F1")
                act(F1[:], A8[:], AF.Exp, scale=32.0)
                # fold fcj: F1 *= qq2 (2x: qq2 read as packed pairs)
                F1v = F1[:].rearrange("p (n h j) -> p n h j", h=4, j=2)
                qqv = qq_sl.rearrange("p (n j) -> p n j", j=2)
                qqb = qqv[:, :, None, :].to_broadcast([128, N, 4, 2])
                nc.vector.tensor_tensor(F1v, F1v, qqb, AL.mult)

                # F2d[n, a, j] = 2*f2[a] duplicated pairwise (j in {0,1});
                # reads G in [a, n] layout (strided input is free on ACT)
                F2d = pF2.tile([128, 16 * N], F16, tag="F2d")
                F2dv = F2d[:].rearrange("p (n a j) -> p n a j", a=8, j=2)
                gna = g_sl.rearrange("p (a n) -> p n a", a=8)
                gb = gna[:, :, :, None].to_broadcast([128, N, 8, 2])
                act(F2dv, gb, AF.Exp, bias=b_ln2, scale=-2.0)

                OUT = outp.tile([128, 64 * N], F16, tag="OUT")
                OUTv = OUT[:].rearrange("p (n a s4 j) -> p n a s4 j", a=8, s4=4, j=2)
                F1j = F1[:].rearrange("p (n s4 j) -> p n s4 j", s4=4, j=2)
                for s4 in range(4):
                    F1b = F1j[:, :, None, s4, :].to_broadcast([128, N, 8, 2])
                    nc.vector.tensor_tensor(OUTv[:, :, :, s4, :], F1b, F2dv,
                                            AL.mult)

                nc.sync.dma_start(
                    out[base: base + TILE_PAIRS, :].rearrange("(p n) f -> p (n f)", p=128),
                    OUT[:],
                )

    # The table-load pass greedily binds each activation fn to the FIRST set
    # containing it. Restrict membership so each phase's functions resolve to
    # one set: Square/Sqrt -> sqrt set (phase A), Sin -> trig (phase B),
    # Ln/Exp -> natural_log_exp (phase C). 3 loads per group, no thrash.
    import concourse.bacc as bacc_mod
    from concourse.hw_specs import get_activation_tables as _real_gat
    keep = {"sqrt_and_others", "trig_and_small", "natural_log_exp_and_others"}

    def _gat(arch):
        return {k: (v if k in keep else set()) for k, v in _real_gat(arch).items()}

    bacc_mod.get_activation_tables = _gat
    try:
        nc.compile()
    finally:
        bacc_mod.get_activation_tables = _real_gat
    return nc


def _cst_array() -> np.ndarray:
    row = np.concatenate([
        (0.475 * np.cos(SHFZ)).astype(np.float32),
        (0.5 * np.sin(SHFZ)).astype(np.float32),
        (2.0 * SHFA).astype(np.float32),
    ])
    return np.broadcast_to(row, (128, 24)).copy()


def _run(vectors12: np.ndarray, trace: bool = False):
    if "nc" not in _CACHE:
        _CACHE["nc"] = _build_nc()
    nc = _CACHE["nc"]

    v = np.ascontiguousarray(np.asarray(vectors12, dtype=np.float32))
    pad = np.zeros((2, P_PAD - P_CORE, 3), np.float32)
    pad[:, :, 0] = 1.0  # unit vectors: all downstream math well-defined
    cst = _cst_array()

    in_maps = []
    for i in range(NCORES):
        shard = v[:, i * P_CORE: (i + 1) * P_CORE, :]
        shard = np.concatenate([shard, pad], axis=1)
        in_maps.append({"vectors12": np.ascontiguousarray(shard), "cst": cst})

    res = run_bass_kernel_spmd(nc, in_maps, core_ids=list(range(NCORES)),
                               trace=trace)
    out = np.empty((P_TOTAL, 64), np.float32)
    for i in range(NCORES):
        shard_out = np.asarray(res.results[i]["out"])[:P_CORE]
        out[i * P_CORE: (i + 1) * P_CORE] = shard_out.astype(np.float32)
    return out, res


def kernel(vectors12, EtaA=None, Zeta=None, ShfA=None, ShfZ=None):
    out, _ = _run(vectors12, trace=False)
    return out


# revision 6
# speedup vs baseline: 1.1889x; 1.1718x over previous
"""Trainium2 Bass kernel for AngularTerms: out[p, a*8+s] = 2*f1[p,s]*f2[p,a]*fcj[p].

Self-contained: hardcodes shapes for vectors12 (2, 2000000, 3) f32 -> (2000000, 64) f32.
Data-parallel over the pair axis P across 8 NeuronCores; no collectives.

Math (per pair p, with v0, v1 the two displacement vectors):
  d_i   = |v_i|
  c     = dot(v0,v1) / (d0*d1)                (clamp is a no-op for this data)
  x     = 0.95*c = cos(theta);  y = sqrt(1 - x^2) = sin(theta)
  f1[s] = ((1 + x*cos(ShfZ_s) + y*sin(ShfZ_s)) / 2) ** 32     (angle-addition; no arccos)
  f2[a] = exp(-8*(h - ShfA_a)^2),  h = (d0+d1)/2
  fcj   = prod_i (0.5*cos(pi*d_i/3.5)+0.5) = (sin(pi/2 - pi*d0/7) * sin(pi/2 - pi*d1/7))^2
  out[p, a*8+s] = 2 * f1[s] * f2[a] * fcj

v4 engine allocation (fp16 wide path, DVE 2x packed):
  - ngroups=1, 3 ACT table loads total.  Square is re-bound to the ln-exp set
    (VV^2 runs on the otherwise-idle gpsimd), so G=(s01-2*ShfA_a)^2 and the
    fcj^2 dup live in phase C with Ln/Exp: nothing is carried but per-pair
    f32 scalars (d0,d1,c,y,s01,q).
  - G via 8 narrow ACT Squares with the per-a constant as bias.
  - ACT instructions carry a linear dep chain: phases never interleave on the
    ACT queue, so each table set loads exactly once.
  - f2 expansion pairwise-duplicated (even tiles, 4 output multiplies over
    s-pairs) vs quad-duplicated (odd tiles, 2 multiplies over s-halves):
    in-kernel A/B experiment for the short-run DVE write cost.
"""
import sys

sys.path.insert(0, "/opt/trn_rl_repo")

import numpy as np
from contextlib import ExitStack

import concourse.bass as bass
import concourse.tile as tile
from concourse import bacc, mybir
from concourse.bass_utils import run_bass_kernel_spmd

F32 = mybir.dt.float32
F16 = mybir.dt.float16
AL = mybir.AluOpType
AF = mybir.ActivationFunctionType

P_TOTAL = 2_000_000
NCORES = 8
P_CORE = P_TOTAL // NCORES      # 250,000
N = 196                          # pairs per partition per tile
T = 10                           # tiles per core
P_PAD = 128 * N * T              # 250,880
CUTOFF = 3.5

SHFA = np.array([0.9, 1.225, 1.55, 1.875, 2.2, 2.525, 2.85, 3.175], np.float32)
SHFZ = np.array([0.19634954, 0.58904862, 0.9817477, 1.37444679,
                 1.76714587, 2.15984495, 2.55254403, 2.94524311], np.float32)

_CACHE: dict = {}


def _build_nc(N=N, T=T, out_bufs=2, act_chain=True, quad_tiles="odd"):
    P_PAD = 128 * N * T
    TILE_PAIRS = 128 * N
    nc = bacc.Bacc()
    vec = nc.declare_dram_parameter("vectors12", [2, P_PAD, 3], F32, isOutput=False)
    cst = nc.declare_dram_parameter("cst", [128, 24], F32, isOutput=False)
    out = nc.declare_dram_parameter("out", [P_PAD, 64], F16, isOutput=True)

    from concourse.bass import _add_dep_helper
    prev_act = [None]

    def act(*args, **kw):
        ins = nc.scalar.activation(*args, **kw)
        if act_chain and prev_act[0] is not None:
            _add_dep_helper(ins.ins, prev_act[0].ins, sync=False,
                            reason="act stream order")
        prev_act[0] = ins
        return ins

    def is_quad(tl):
        if quad_tiles == "odd":
            return tl % 2 == 1
        return quad_tiles == "all"

    with tile.TileContext(nc) as tc, ExitStack() as ctx:
        const = ctx.enter_context(tc.tile_pool(name="const", bufs=1))
        carp = ctx.enter_context(tc.tile_pool(name="car", bufs=1))
        pA = ctx.enter_context(tc.tile_pool(name="pA", bufs=2))
        tmpA = ctx.enter_context(tc.tile_pool(name="tmpA", bufs=3))
        pB = ctx.enter_context(tc.tile_pool(name="pB", bufs=2))
        pC = ctx.enter_context(tc.tile_pool(name="pC", bufs=2))
        pG = ctx.enter_context(tc.tile_pool(name="pG", bufs=2))
        pF1 = ctx.enter_context(tc.tile_pool(name="pF1", bufs=3))
        pF2 = ctx.enter_context(tc.tile_pool(name="pF2", bufs=2))
        outp = ctx.enter_context(tc.tile_pool(name="outp", bufs=out_bufs))

        cstT = const.tile([128, 24], F32)
        nc.sync.dma_start(cstT[:], cst[:])
        CA = cstT[:, 0:8]     # 0.475*cos(ShfZ)
        SA = cstT[:, 8:16]    # 0.5*sin(ShfZ)

        def const_scalar(val, name):
            t = const.tile([128, 1], F32, tag=name)
            nc.vector.memset(t[:], float(val))
            return t[:]

        b_pi2 = const_scalar(np.pi / 2, "pi2")
        b_half = const_scalar(0.5, "half")
        b_ln2 = const_scalar(float(np.log(2.0)), "ln2")
        b_one = const_scalar(1.0, "one")
        b_shfa = [const_scalar(-2.0 * float(SHFA[a]), f"shfa{a}") for a in range(8)]

        # carried per-tile f32 scalars: d0,d1 (2N) | c | y | s01 | q
        CARW = 6 * N
        carf = carp.tile([128, CARW * T], F32, tag="carf")

        def slots(tl):
            b = tl * CARW
            return (carf[:, b: b + 2 * N],            # d0,d1
                    carf[:, b + 2 * N: b + 3 * N],    # c
                    carf[:, b + 3 * N: b + 4 * N],    # y
                    carf[:, b + 4 * N: b + 5 * N],    # s01
                    carf[:, b + 5 * N: b + 6 * N])    # q = cos(pi d0/7) cos(pi d1/7)

        # ---------- Phase A: squares, norms, c, y (sqrt table set) ----------
        for tl in range(T):
            base = tl * TILE_PAIRS
            d_sl, c_sl, y_sl, s01_sl, _ = slots(tl)

            VV = pA.tile([128, 6 * N], F32, tag="VV")
            nc.sync.dma_start(
                VV[:, : 3 * N],
                vec[0, base: base + TILE_PAIRS, :].rearrange("(p n) c -> p (n c)", p=128),
            )
            nc.sync.dma_start(
                VV[:, 3 * N:],
                vec[1, base: base + TILE_PAIRS, :].rearrange("(p n) c -> p (n c)", p=128),
            )
            # W = [v0^2 (3N) | v1^2 (3N) | v0*v1 (3N)]; the squares run on
            # gpsimd so ACT phase A stays pure-sqrt (Square binds to ln-exp)
            W = pA.tile([128, 9 * N], F32, tag="W")
            nc.gpsimd.tensor_tensor(W[:, : 6 * N], VV[:], VV[:], AL.mult)
            nc.vector.tensor_tensor(W[:, 6 * N:], VV[:, : 3 * N], VV[:, 3 * N:],
                                    AL.mult)
            W3 = W[:].rearrange("p (k n c) -> p k n c", k=3, c=3)
            D3 = pA.tile([128, 3 * N], F32, tag="D3")
            D3v = D3[:].rearrange("p (k n) -> p k n", k=3)
            nc.vector.tensor_tensor(D3v, W3[:, :, :, 0], W3[:, :, :, 1], AL.add)
            nc.vector.tensor_tensor(D3v, D3v, W3[:, :, :, 2], AL.add)
            # D3 = [d0^2 | d1^2 | dot]
            act(d_sl, D3[:, : 2 * N], AF.Sqrt)

            nc.vector.tensor_tensor(s01_sl, d_sl[:, :N], d_sl[:, N:], AL.add)
            m = tmpA.tile([128, N], F32, tag="m")
            nc.vector.tensor_tensor(m[:], d_sl[:, :N], d_sl[:, N:], AL.mult)
            rm = tmpA.tile([128, N], F32, tag="rm")
            nc.vector.reciprocal_approx_fast(rm[:], m[:])
            nc.vector.tensor_tensor(c_sl, D3[:, 2 * N:], rm[:], AL.mult)

            # cc = -0.9025 c^2 (scale folded);  y = sqrt(cc + 1) = sin(theta)
            cc = tmpA.tile([128, N], F32, tag="cc")
            nc.vector.scalar_tensor_tensor(
                cc[:], c_sl, -0.9025, c_sl, AL.mult, AL.mult)
            act(y_sl, cc[:], AF.Sqrt, bias=b_one)

        # ---------- Phase B: fcj via sin (trig table set) -------------------
        for tl in range(T):
            d_sl, _, _, _, q_sl = slots(tl)
            S12 = pB.tile([128, 2 * N], F32, tag="S12")
            # sin(pi/2 - (pi/7) d) = cos(pi d / 7);  fcj_i = cos^2(pi d_i/7)
            act(S12[:], d_sl, AF.Sin, bias=b_pi2, scale=float(-np.pi / 7))
            nc.vector.tensor_tensor(q_sl, S12[:, :N], S12[:, N:], AL.mult)

        # ---------- Phase C: f1, f2, outer product (ln+exp set) -------------
        for tl in range(T):
            base = tl * TILE_PAIRS
            _, c_sl, y_sl, s01_sl, q_sl = slots(tl)

            # qq2[n, j] = fcj duplicated pair (fp16) for the 2x F1 fold
            qq2 = pG.tile([128, 2 * N], F16, tag="qq2")
            qq2v = qq2[:].rearrange("p (n j) -> p n j", j=2)
            qb = q_sl[:, :, None].to_broadcast([128, N, 2])
            act(qq2v, qb, AF.Square)

            # G[a, n] = (s01 - 2*ShfA_a)^2 via 8 narrow Squares (bias = per-a)
            G = pG.tile([128, 8 * N], F16, tag="G")
            for a in range(8):
                act(G[:, a * N: (a + 1) * N], s01_sl, AF.Square, bias=b_shfa[a])

            A8 = pC.tile([128, 8 * N], F32, tag="A8")
            B8 = pC.tile([128, 8 * N], F32, tag="B8")
            A8v = A8[:].rearrange("p (n s) -> p n s", s=8)
            B8v = B8[:].rearrange("p (n s) -> p n s", s=8)
            cb = c_sl[:, :, None].to_broadcast([128, N, 8])
            yb = y_sl[:, :, None].to_broadcast([128, N, 8])
            CAb = CA[:, None, :].to_broadcast([128, N, 8])
            SAb = SA[:, None, :].to_broadcast([128, N, 8])
            nc.vector.tensor_tensor(A8v, CAb, cb, AL.mult)
            nc.vector.tensor_tensor(B8v, SAb, yb, AL.mult)
            nc.vector.tensor_tensor(A8[:], A8[:], B8[:], AL.add)
            # lt = ln(x*ca + y*sa + 0.5); f1 = exp(32*lt) = t^32
            act(A8[:], A8[:], AF.Ln, bias=b_half)
            F1 = pF1.tile([128, 8 * N], F16, tag="F1")
            act(F1[:], A8[:], AF.Exp, scale=32.0)
            # fold fcj: F1 *= qq2 (2x: qq2 read as packed pairs)
            F1v = F1[:].rearrange("p (n h j) -> p n h j", h=4, j=2)
            qqb = qq2v[:, :, None, :].to_broadcast([128, N, 4, 2])
            nc.vector.tensor_tensor(F1v, F1v, qqb, AL.mult)

            gna = G[:].rearrange("p (a n) -> p n a", a=8)
            OUT = outp.tile([128, 64 * N], F16, tag="OUT")
            if not is_quad(tl):
                # pair-dup: F2d[n, a, j], 4 multiplies over s-pairs
                F2d = pF2.tile([128, 32 * N], F16, tag="F2x")
                F2dv = F2d[:, : 16 * N].rearrange("p (n a j) -> p n a j", a=8, j=2)
                gb = gna[:, :, :, None].to_broadcast([128, N, 8, 2])
                act(F2dv, gb, AF.Exp, bias=b_ln2, scale=-2.0)
                OUTv = OUT[:].rearrange("p (n a s4 j) -> p n a s4 j", a=8, s4=4, j=2)
                F1j = F1[:].rearrange("p (n s4 j) -> p n s4 j", s4=4, j=2)
                for s4 in range(4):
                    F1b = F1j[:, :, None, s4, :].to_broadcast([128, N, 8, 2])
                    nc.vector.tensor_tensor(OUTv[:, :, :, s4, :], F1b, F2dv,
                                            AL.mult)
            else:
                # quad-dup: F2q[n, a, j4], 2 multiplies over s-halves
                F2q = pF2.tile([128, 32 * N], F16, tag="F2x")
                F2qv = F2q[:].rearrange("p (n a j) -> p n a j", a=8, j=4)
                gb = gna[:, :, :, None].to_broadcast([128, N, 8, 4])
                act(F2qv, gb, AF.Exp, bias=b_ln2, scale=-2.0)
                OUTv = OUT[:].rearrange("p (n a h j) -> p n a h j", a=8, h=2, j=4)
                F1h = F1[:].rearrange("p (n h j) -> p n h j", h=2, j=4)
                for h in range(2):
                    F1b = F1h[:, :, None, h, :].to_broadcast([128, N, 8, 4])
                    nc.vector.tensor_tensor(OUTv[:, :, :, h, :], F1b, F2qv,
                                            AL.mult)

            nc.sync.dma_start(
                out[base: base + TILE_PAIRS, :].rearrange("(p n) f -> p (n f)", p=128),
                OUT[:],
            )

    # Bind each activation fn to exactly one kept set: Sqrt -> sqrt set
    # (phase A), Sin -> trig (phase B), Square/Ln/Exp -> natural_log_exp
    # (phase C).  With the linear ACT chain: 3 table loads total.
    import concourse.bacc as bacc_mod
    from concourse.hw_specs import get_activation_tables as _real_gat
    keep = {"sqrt_and_others", "trig_and_small", "natural_log_exp_and_others"}

    def _gat(arch):
        t = {}
        for k, v in _real_gat(arch).items():
            if k not in keep:
                t[k] = set()
                continue
            v = set(v)
            if k != "natural_log_exp_and_others":
                v.discard(AF.Square)
            t[k] = v
        return t

    bacc_mod.get_activation_tables = _gat
    try:
        nc.compile()
    finally:
        bacc_mod.get_activation_tables = _real_gat
    return nc


def _cst_array() -> np.ndarray:
    row = np.concatenate([
        (0.475 * np.cos(SHFZ)).astype(np.float32),
        (0.5 * np.sin(SHFZ)).astype(np.float32),
        (2.0 * SHFA).astype(np.float32),
    ])
    return np.broadcast_to(row, (128, 24)).copy()


def _run(vectors12: np.ndarray, trace: bool = False):
    if "nc" not in _CACHE:
        _CACHE["nc"] = _build_nc()
    nc = _CACHE["nc"]

    v = np.ascontiguousarray(np.asarray(vectors12, dtype=np.float32))
    pad = np.zeros((2, P_PAD - P_CORE, 3), np.float32)
    pad[:, :, 0] = 1.0  # unit vectors: all downstream math well-defined
    cst = _cst_array()

    in_maps = []
    for i in range(NCORES):
        shard = v[:, i * P_CORE: (i + 1) * P_CORE, :]
        shard = np.concatenate([shard, pad], axis=1)
        in_maps.append({"vectors12": np.ascontiguousarray(shard), "cst": cst})

    res = run_bass_kernel_spmd(nc, in_maps, core_ids=list(range(NCORES)),
                               trace=trace)
    out = np.empty((P_TOTAL, 64), np.float32)
    for i in range(NCORES):
        shard_out = np.asarray(res.results[i]["out"])[:P_CORE]
        out[i * P_CORE: (i + 1) * P_CORE] = shard_out.astype(np.float32)
    return out, res


def kernel(vectors12, EtaA=None, Zeta=None, ShfA=None, ShfZ=None):
    out, _ = _run(vectors12, trace=False)
    return out


# revision 7
# speedup vs baseline: 1.2720x; 1.0699x over previous
"""Trainium2 Bass kernel for AngularTerms: out[p, a*8+s] = 2*f1[p,s]*f2[p,a]*fcj[p].

Self-contained: hardcodes shapes for vectors12 (2, 2000000, 3) f32 -> (2000000, 64) f32.
Data-parallel over the pair axis P across 8 NeuronCores; no collectives.

Math (per pair p, with v0, v1 the two displacement vectors):
  d_i   = |v_i|
  c     = dot(v0,v1) / (d0*d1)                (clamp is a no-op for this data)
  x     = 0.95*c = cos(theta);  y = sqrt(1 - x^2) = sin(theta)
  f1[s] = ((1 + x*cos(ShfZ_s) + y*sin(ShfZ_s)) / 2) ** 32     (angle-addition; no arccos)
  f2[a] = exp(-8*(h - ShfA_a)^2),  h = (d0+d1)/2
  fcj   = prod_i (0.5*cos(pi*d_i/3.5)+0.5) = (sin(pi/2 - pi*d0/7) * sin(pi/2 - pi*d1/7))^2
  out[p, a*8+s] = 2 * f1[s] * f2[a] * fcj

v4 engine allocation (fp16 wide path, DVE 2x packed):
  - ngroups=1, 3 ACT table loads total.  Square is re-bound to the ln-exp set
    (VV^2 runs on the otherwise-idle gpsimd), so G=(s01-2*ShfA_a)^2 and the
    fcj^2 dup live in phase C with Ln/Exp: nothing is carried but per-pair
    f32 scalars (d0,d1,c,y,s01,q).
  - G via 8 narrow ACT Squares with the per-a constant as bias.
  - ACT instructions carry a linear dep chain: phases never interleave on the
    ACT queue, so each table set loads exactly once.
  - f2 expansion pairwise-duplicated (even tiles, 4 output multiplies over
    s-pairs) vs quad-duplicated (odd tiles, 2 multiplies over s-halves):
    in-kernel A/B experiment for the short-run DVE write cost.
"""
import sys

sys.path.insert(0, "/opt/trn_rl_repo")

import numpy as np
from contextlib import ExitStack

import concourse.bass as bass
import concourse.tile as tile
from concourse import bacc, mybir
from concourse.bass_utils import run_bass_kernel_spmd

F32 = mybir.dt.float32
F16 = mybir.dt.float16
AL = mybir.AluOpType
AF = mybir.ActivationFunctionType

P_TOTAL = 2_000_000
NCORES = 8
P_CORE = P_TOTAL // NCORES      # 250,000
N = 196                          # pairs per partition per tile
T = 10                           # tiles per core
P_PAD = 128 * N * T              # 250,880
CUTOFF = 3.5

SHFA = np.array([0.9, 1.225, 1.55, 1.875, 2.2, 2.525, 2.85, 3.175], np.float32)
SHFZ = np.array([0.19634954, 0.58904862, 0.9817477, 1.37444679,
                 1.76714587, 2.15984495, 2.55254403, 2.94524311], np.float32)

_CACHE: dict = {}


def _build_nc(N=N, T=T, out_bufs=2, act_chain=True, quad_tiles="odd"):
    P_PAD = 128 * N * T
    TILE_PAIRS = 128 * N
    nc = bacc.Bacc()
    vec = nc.declare_dram_parameter("vectors12", [2, P_PAD, 3], F32, isOutput=False)
    cst = nc.declare_dram_parameter("cst", [128, 24], F32, isOutput=False)
    out = nc.declare_dram_parameter("out", [P_PAD, 64], F16, isOutput=True)

    from concourse.bass import _add_dep_helper
    prev_act = [None]

    def act(*args, **kw):
        ins = nc.scalar.activation(*args, **kw)
        if act_chain and prev_act[0] is not None:
            _add_dep_helper(ins.ins, prev_act[0].ins, sync=False,
                            reason="act stream order")
        prev_act[0] = ins
        return ins

    def is_quad(tl):
        if quad_tiles == "odd":
            return tl % 2 == 1
        return quad_tiles == "all"

    with tile.TileContext(nc) as tc, ExitStack() as ctx:
        const = ctx.enter_context(tc.tile_pool(name="const", bufs=1))
        carp = ctx.enter_context(tc.tile_pool(name="car", bufs=1))
        pA = ctx.enter_context(tc.tile_pool(name="pA", bufs=2))
        tmpA = ctx.enter_context(tc.tile_pool(name="tmpA", bufs=3))
        pB = ctx.enter_context(tc.tile_pool(name="pB", bufs=2))
        pC = ctx.enter_context(tc.tile_pool(name="pC", bufs=2))
        pG = ctx.enter_context(tc.tile_pool(name="pG", bufs=2))
        pF1 = ctx.enter_context(tc.tile_pool(name="pF1", bufs=3))
        pF2 = ctx.enter_context(tc.tile_pool(name="pF2", bufs=2))
        outp = ctx.enter_context(tc.tile_pool(name="outp", bufs=out_bufs))

        cstT = const.tile([128, 24], F32)
        nc.sync.dma_start(cstT[:], cst[:])
        CA = cstT[:, 0:8]     # 0.475*cos(ShfZ)
        SA = cstT[:, 8:16]    # 0.5*sin(ShfZ)

        def const_scalar(val, name):
            t = const.tile([128, 1], F32, tag=name)
            nc.vector.memset(t[:], float(val))
            return t[:]

        b_pi2 = const_scalar(np.pi / 2, "pi2")
        b_half = const_scalar(0.5, "half")
        b_ln2 = const_scalar(float(np.log(2.0)), "ln2")
        b_one = const_scalar(1.0, "one")
        b_shfa = [const_scalar(-2.0 * float(SHFA[a]), f"shfa{a}") for a in range(8)]

        # carried per-tile f32 scalars: d0,d1 (2N) | c | y | s01 | q
        CARW = 6 * N
        carf = carp.tile([128, CARW * T], F32, tag="carf")

        def slots(tl):
            b = tl * CARW
            return (carf[:, b: b + 2 * N],            # d0,d1
                    carf[:, b + 2 * N: b + 3 * N],    # c
                    carf[:, b + 3 * N: b + 4 * N],    # y
                    carf[:, b + 4 * N: b + 5 * N],    # s01
                    carf[:, b + 5 * N: b + 6 * N])    # q = cos(pi d0/7) cos(pi d1/7)

        # ---------- Phase A: squares, norms, c, y (sqrt table set) ----------
        for tl in range(T):
            base = tl * TILE_PAIRS
            d_sl, c_sl, y_sl, s01_sl, _ = slots(tl)

            VV = pA.tile([128, 6 * N], F32, tag="VV")
            nc.sync.dma_start(
                VV[:, : 3 * N],
                vec[0, base: base + TILE_PAIRS, :].rearrange("(p n) c -> p (n c)", p=128),
            )
            nc.sync.dma_start(
                VV[:, 3 * N:],
                vec[1, base: base + TILE_PAIRS, :].rearrange("(p n) c -> p (n c)", p=128),
            )
            # W = [v0^2 (3N) | v1^2 (3N) | v0*v1 (3N)]; the squares run on
            # gpsimd so ACT phase A stays pure-sqrt (Square binds to ln-exp)
            W = pA.tile([128, 9 * N], F32, tag="W")
            nc.gpsimd.tensor_tensor(W[:, : 6 * N], VV[:], VV[:], AL.mult)
            nc.vector.tensor_tensor(W[:, 6 * N:], VV[:, : 3 * N], VV[:, 3 * N:],
                                    AL.mult)
            W3 = W[:].rearrange("p (k n c) -> p k n c", k=3, c=3)
            D3 = pA.tile([128, 3 * N], F32, tag="D3")
            D3v = D3[:].rearrange("p (k n) -> p k n", k=3)
            nc.vector.tensor_tensor(D3v, W3[:, :, :, 0], W3[:, :, :, 1], AL.add)
            nc.vector.tensor_tensor(D3v, D3v, W3[:, :, :, 2], AL.add)
            # D3 = [d0^2 | d1^2 | dot]
            act(d_sl, D3[:, : 2 * N], AF.Sqrt)

            nc.vector.tensor_tensor(s01_sl, d_sl[:, :N], d_sl[:, N:], AL.add)
            m = tmpA.tile([128, N], F32, tag="m")
            nc.vector.tensor_tensor(m[:], d_sl[:, :N], d_sl[:, N:], AL.mult)
            rm = tmpA.tile([128, N], F32, tag="rm")
            nc.vector.reciprocal_approx_fast(rm[:], m[:])
            nc.vector.tensor_tensor(c_sl, D3[:, 2 * N:], rm[:], AL.mult)

            # cc = -0.9025 c^2 (scale folded);  y = sqrt(cc + 1) = sin(theta)
            cc = tmpA.tile([128, N], F32, tag="cc")
            nc.vector.scalar_tensor_tensor(
                cc[:], c_sl, -0.9025, c_sl, AL.mult, AL.mult)
            act(y_sl, cc[:], AF.Sqrt, bias=b_one)

        # ---------- Phase B: fcj via sin (trig table set) -------------------
        for tl in range(T):
            d_sl, _, _, _, q_sl = slots(tl)
            S12 = pB.tile([128, 2 * N], F32, tag="S12")
            # sin(pi/2 - (pi/7) d) = cos(pi d / 7);  fcj_i = cos^2(pi d_i/7)
            act(S12[:], d_sl, AF.Sin, bias=b_pi2, scale=float(-np.pi / 7))
            nc.vector.tensor_tensor(q_sl, S12[:, :N], S12[:, N:], AL.mult)

        # ---------- Phase C: f1, f2, outer product (ln+exp set) -------------
        # Software-pipelined one stage: the F1 fold + outer product + store
        # for tile t-1 are emitted after A8/B8/T8 of tile t, so the DVE never
        # stalls on the ACT Ln->Exp chain.
        pend = [None]

        def flush_pend():
            if pend[0] is None:
                return
            base, F1, qq2v, F2qv = pend[0]
            pend[0] = None
            # fold fcj: F1 *= qq2 (2x: qq2 read as packed pairs)
            F1v = F1[:].rearrange("p (n h j) -> p n h j", h=4, j=2)
            qqb = qq2v[:, :, None, :].to_broadcast([128, N, 4, 2])
            nc.vector.tensor_tensor(F1v, F1v, qqb, AL.mult)
            OUT = outp.tile([128, 64 * N], F16, tag="OUT")
            OUTv = OUT[:].rearrange("p (n a h j) -> p n a h j", a=8, h=2, j=4)
            F1h = F1[:].rearrange("p (n h j) -> p n h j", h=2, j=4)
            for h in range(2):
                F1b = F1h[:, :, None, h, :].to_broadcast([128, N, 8, 4])
                nc.vector.tensor_tensor(OUTv[:, :, :, h, :], F1b, F2qv, AL.mult)
            nc.sync.dma_start(
                out[base: base + TILE_PAIRS, :].rearrange("(p n) f -> p (n f)", p=128),
                OUT[:],
            )

        for tl in range(T):
            base = tl * TILE_PAIRS
            _, c_sl, y_sl, s01_sl, q_sl = slots(tl)

            # qq2[n, j] = fcj duplicated pair (fp16) for the 2x F1 fold
            qq2 = pG.tile([128, 2 * N], F16, tag="qq2")
            qq2v = qq2[:].rearrange("p (n j) -> p n j", j=2)
            qb = q_sl[:, :, None].to_broadcast([128, N, 2])
            act(qq2v, qb, AF.Square)

            # G[a, n] = (s01 - 2*ShfA_a)^2 via 8 narrow Squares (bias = per-a)
            G = pG.tile([128, 8 * N], F16, tag="G")
            for a in range(8):
                act(G[:, a * N: (a + 1) * N], s01_sl, AF.Square, bias=b_shfa[a])

            A8 = pC.tile([128, 8 * N], F32, tag="A8")
            B8 = pC.tile([128, 8 * N], F32, tag="B8")
            A8v = A8[:].rearrange("p (n s) -> p n s", s=8)
            B8v = B8[:].rearrange("p (n s) -> p n s", s=8)
            cb = c_sl[:, :, None].to_broadcast([128, N, 8])
            yb = y_sl[:, :, None].to_broadcast([128, N, 8])
            CAb = CA[:, None, :].to_broadcast([128, N, 8])
            SAb = SA[:, None, :].to_broadcast([128, N, 8])
            nc.vector.tensor_tensor(A8v, CAb, cb, AL.mult)
            nc.vector.tensor_tensor(B8v, SAb, yb, AL.mult)
            nc.vector.tensor_tensor(A8[:], A8[:], B8[:], AL.add)
            # lt = ln(x*ca + y*sa + 0.5); f1 = exp(32*lt) = t^32
            act(A8[:], A8[:], AF.Ln, bias=b_half)
            F1 = pF1.tile([128, 8 * N], F16, tag="F1")
            act(F1[:], A8[:], AF.Exp, scale=32.0)

            # quad-dup: F2q[n, a, j4] = 2*f2[a] x4
            gna = G[:].rearrange("p (a n) -> p n a", a=8)
            F2q = pF2.tile([128, 32 * N], F16, tag="F2x")
            F2qv = F2q[:].rearrange("p (n a j) -> p n a j", a=8, j=4)
            gb = gna[:, :, :, None].to_broadcast([128, N, 8, 4])
            act(F2qv, gb, AF.Exp, bias=b_ln2, scale=-2.0)

            flush_pend()
            pend[0] = (base, F1, qq2v, F2qv)
        flush_pend()

    # Bind each activation fn to exactly one kept set: Sqrt -> sqrt set
    # (phase A), Sin -> trig (phase B), Square/Ln/Exp -> natural_log_exp
    # (phase C).  With the linear ACT chain: 3 table loads total.
    import concourse.bacc as bacc_mod
    from concourse.hw_specs import get_activation_tables as _real_gat
    keep = {"sqrt_and_others", "trig_and_small", "natural_log_exp_and_others"}

    def _gat(arch):
        t = {}
        for k, v in _real_gat(arch).items():
            if k not in keep:
                t[k] = set()
                continue
            v = set(v)
            if k != "natural_log_exp_and_others":
                v.discard(AF.Square)
            t[k] = v
        return t

    bacc_mod.get_activation_tables = _gat
    try:
        nc.compile()
    finally:
        bacc_mod.get_activation_tables = _real_gat
    return nc


def _cst_array() -> np.ndarray:
    row = np.concatenate([
        (0.475 * np.cos(SHFZ)).astype(np.float32),
        (0.5 * np.sin(SHFZ)).astype(np.float32),
        (2.0 * SHFA).astype(np.float32),
    ])
    return np.broadcast_to(row, (128, 24)).copy()


def _run(vectors12: np.ndarray, trace: bool = False):
    if "nc" not in _CACHE:
        _CACHE["nc"] = _build_nc()
    nc = _CACHE["nc"]

    v = np.ascontiguousarray(np.asarray(vectors12, dtype=np.float32))
    pad = np.zeros((2, P_PAD - P_CORE, 3), np.float32)
    pad[:, :, 0] = 1.0  # unit vectors: all downstream math well-defined
    cst = _cst_array()

    in_maps = []
    for i in range(NCORES):
        shard = v[:, i * P_CORE: (i + 1) * P_CORE, :]
        shard = np.concatenate([shard, pad], axis=1)
        in_maps.append({"vectors12": np.ascontiguousarray(shard), "cst": cst})

    res = run_bass_kernel_spmd(nc, in_maps, core_ids=list(range(NCORES)),
                               trace=trace)
    out = np.empty((P_TOTAL, 64), np.float32)
    for i in range(NCORES):
        shard_out = np.asarray(res.results[i]["out"])[:P_CORE]
        out[i * P_CORE: (i + 1) * P_CORE] = shard_out.astype(np.float32)
    return out, res


def kernel(vectors12, EtaA=None, Zeta=None, ShfA=None, ShfZ=None):
    out, _ = _run(vectors12, trace=False)
    return out
